# revision 1
# baseline (speedup 1.0000x reference)
"""Trainium2 Bass kernel for nn_BatchMatMulModule.

Computes out = einsum("bnij,bmj->bnmi", x, y) with
  x: [4, 64, 3, 3] f32, y: [4, 100000, 3] f32 -> out: [4, 64, 100000, 3] f32.

The output (307 MB) dwarfs the inputs (4.8 MB), so the kernel is bound by
HBM write bandwidth (~358 GB/s per NeuronCore). Strategy:

- Shard the 256 flat (b, n) pairs across 8 cores: core c handles b = c // 2
  and n in [32 * (c % 2), 32 * (c % 2) + 32). Each core's output slice
  [32, 100000, 3] is a contiguous chunk of the full output.
- Per core, SBUF layout packs partition dim = (n_sub in 0..4, m_segment in
  0..32) and free dim = (3125 rows x 3 output cols). Each partition's free
  segment is a 9375-float contiguous run of the output, so store-DMAs move
  4.8 MB with 37.5 KB contiguous per partition (near peak DMA efficiency).
- Compute is elementwise: out[:, t, i] = sum_j y[:, t, j] * x[n, i, j].
  y is resident in SBUF (replicated over the 4 n_sub partition groups);
  x values are per-partition scalars. The j = 0 term runs on the Scalar
  engine (activation Copy with per-partition scale); the j = 1, 2 terms are
  fused multiply-accumulates (scalar_tensor_tensor) on Vector/GPSIMD.
"""

import numpy as np

import concourse.bacc as bacc
import concourse.mybir as mybir
from concourse.bass_utils import run_bass_kernel_spmd
from concourse.tile import TileContext

N_CORES = 8
P = 128
N_PER_CORE = 32  # (b, n) pairs per core
N_SUB = 4        # n values packed across partition groups
N_GROUPS = N_PER_CORE // N_SUB  # 8 groups, one out tile each
SEGS = P // N_SUB               # 32 m-segments per n
M = 100000
ROWS = M // SEGS                # 3125 rows per partition
FREE = ROWS * 3                 # 9375 f32 per partition

TRACE = False
LAST = None  # last BassKernelResults, for test harness introspection

_CACHED_NC = None


# Engine routing/tiling knobs. HW-tuned: GPSIMD participation consistently
# regressed wall time (scheduler interlock), so accumulates stay on DVE;
# one 4.8 MB store per group; chains staged through contiguous tmp tiles.
POOL_CHAINS_MOD = 1   # fraction LT/MOD of accum ops routed via GPSIMD
POOL_CHAINS_LT = 0
POOL_UNITS = 0        # if >0: this many trailing units run fully on ACT+GPSIMD
DMA_ACCUM_CHAINS = 0  # chains whose j=1 accum runs as a SWDGE CCE-add DMA
OUT_BUFS = 3
TMP_BUFS = 3
SPLITS = 1


def build_bass(reps: int = 1, ops_mode: str = "full"):
    nc = bacc.Bacc(
        "TRN2",
        debug=False,
        enable_asserts=False,
        target_bir_lowering=False,
        num_devices=N_CORES,
    )
    f32 = mybir.dt.float32
    # xs arrives host-pre-expanded to the per-partition scalar layout:
    # xs[p = a*32 + s, col = g*9 + i*3 + j] = x[g*4 + a, i, j].
    xs = nc.dram_tensor("xs", [P, N_GROUPS * 9], f32, kind="ExternalInput").ap()
    ys = nc.dram_tensor("ys", [M, 3], f32, kind="ExternalInput").ap()
    out = nc.dram_tensor("out", [N_PER_CORE, M, 3], f32, kind="ExternalOutput").ap()

    mult = mybir.AluOpType.mult
    add = mybir.AluOpType.add
    copy = mybir.ActivationFunctionType.Copy

    with TileContext(nc) as tc:
        with (
            tc.tile_pool(name="const", bufs=1) as cpool,
            tc.tile_pool(name="outp", bufs=OUT_BUFS) as opool,
            tc.tile_pool(name="tmpp", bufs=TMP_BUFS) as tpool,
            tc.tile_pool(name="psp", bufs=2, space="PSUM") as pspool,
        ):
            # x scalars: partition (a, s) holds x[g*4 + a, i, j] at free
            # index g*9 + i*3 + j.
            xsb = cpool.tile([P, N_GROUPS * 9], f32)
            nc.sync.dma_start(out=xsb[:], in_=xs)

            # Split each group's rows into SPLITS chunks: shorter dependency
            # chains and earlier stores.
            bounds = [ROWS * s // SPLITS for s in range(SPLITS + 1)]

            # y resident in SBUF: partition (a, s) holds y rows
            # [s*ROWS, (s+1)*ROWS) — identical for every a group. Loaded in
            # SPLITS column chunks so first-chunk compute starts early.
            y_tile = cpool.tile([P, FREE], f32)
            y_src3 = ys.rearrange("(s t) i -> s t i", s=SEGS)  # [32, 3125, 3]
            for h in range(SPLITS):
                t0, t1 = bounds[h], bounds[h + 1]
                nc.sync.dma_start(
                    out=y_tile[:, t0 * 3:t1 * 3],
                    in_=y_src3[:, t0:t1, :].rearrange("s t i -> s (t i)")
                    .unsqueeze(0).to_broadcast((N_SUB, SEGS, (t1 - t0) * 3)),
                )

            yv = y_tile.rearrange("p (t i) -> p t i", i=3)
            light_store = ops_mode.endswith("L")
            if light_store:
                ops_mode = ops_mode[:-1]

            units = [(gg % N_GROUPS, h)
                     for gg in range(N_GROUPS * reps) for h in range(SPLITS)]

            def store_unit(g, h, ot):
                t0, t1 = bounds[h], bounds[h + 1]
                if light_store and g >= 1:
                    return
                dst = out[g * N_SUB:(g + 1) * N_SUB, :, :].rearrange(
                    "a (s t) i -> (a s) t i", s=SEGS
                )[:, t0:t1, :]
                nc.sync.dma_start(out=dst, in_=ot[:])

            def pool_assign(g, h, i, j):
                kk = ((g * SPLITS + h) * 3 + i) * 2 + (j - 1)
                return (kk * POOL_CHAINS_LT) % POOL_CHAINS_MOD < POOL_CHAINS_LT

            def emit_products(g, h):
                """ACT products for unit (g, h): j=0 terms into tm tiles,
                plus the products consumed by GPSIMD-routed accumulates."""
                t0, t1 = bounds[h], bounds[h + 1]
                nt = t1 - t0
                yh = yv[:, t0:t1, :]
                tms, tm2s = [], {}
                for i in range(3):
                    c = g * 9 + i * 3
                    tm = tpool.tile([P, nt], f32, name="tm", tag="tm")
                    nc.scalar.activation(
                        out=tm[:], in_=yh[:, :, 0], func=copy,
                        scale=xsb[:, c:c + 1],
                    )
                    tms.append(tm)
                    if (g * SPLITS + h) * 3 + i < DMA_ACCUM_CHAINS:
                        # j=1 product for the CCE-add DMA accumulate path.
                        tq = tpool.tile([P, nt], f32, name="tq", tag="tq",
                                        bufs=2)
                        nc.scalar.activation(
                            out=tq[:], in_=yh[:, :, 1], func=copy,
                            scale=xsb[:, c + 1:c + 2],
                        )
                        tm2s[(i, "dma")] = tq
                    for j in (1, 2):
                        if pool_assign(g, h, i, j):
                            tm2 = tpool.tile([P, nt], f32, name="tm2",
                                             tag="tm2")
                            nc.scalar.activation(
                                out=tm2[:], in_=yh[:, :, j], func=copy,
                                scale=xsb[:, c + j:c + j + 1],
                            )
                            tm2s[(i, j)] = tm2
                return tms, tm2s

            def emit_accums(g, h, tms, tm2s):
                """Accumulates for unit (g, h): two-tensor-input ops run
                ~1.6 ns/elem on DVE (stt), ~2.7 on GPSIMD (tt-add; stt is
                not a legal GPSIMD opcode). Each i-chain accumulates in its
                own contiguous tm tile; the final accumulate writes the
                strided out slice, interleaving for free."""
                t0, t1 = bounds[h], bounds[h + 1]
                nt = t1 - t0
                yh = yv[:, t0:t1, :]
                ot = opool.tile([P, nt * 3], f32, name="ot", tag="ot")
                ov = ot.rearrange("p (t i) -> p t i", i=3)
                for i in range(3):
                    c = g * 9 + i * 3
                    tm = tms[i]
                    srcs = {1: tm, 2: tm}
                    for j in (1, 2):
                        dst_ap = ov[:, :, i] if j == 2 else tm[:]
                        if j == 1 and (i, "dma") in tm2s:
                            # Accumulate on the idle DMA engines: SWDGE
                            # SBUF->SBUF copy with CCE add into tm.
                            nc.gpsimd.dma_start(
                                out=tm[:], in_=tm2s[(i, "dma")][:],
                                accum_op=add,
                            )
                        elif (i, j) in tm2s:
                            nc.gpsimd.tensor_tensor(
                                out=dst_ap, in0=srcs[j][:],
                                in1=tm2s[(i, j)][:], op=add,
                            )
                        else:
                            nc.vector.scalar_tensor_tensor(
                                out=dst_ap, in0=yh[:, :, j],
                                scalar=xsb[:, c + j:c + j + 1],
                                in1=srcs[j][:], op0=mult, op1=add,
                            )
                store_unit(g, h, ot)

            def emit_pool_unit(g, h):
                """A unit computed entirely on ACT (products) + GPSIMD
                (adds), decoupled from the DVE pipeline via dedicated
                pools. Processed in quarter-row chunks to keep the
                dedicated pools small."""
                t0u, t1u = bounds[h], bounds[h + 1]
                qb = [t0u + (t1u - t0u) * q // 4 for q in range(5)]
                for q in range(4):
                    t0, t1 = qb[q], qb[q + 1]
                    nt = t1 - t0
                    yh = yv[:, t0:t1, :]
                    ot = ppool.tile([P, nt * 3], f32, name="pot", tag="pot")
                    ov = ot.rearrange("p (t i) -> p t i", i=3)
                    for i in range(3):
                        c = g * 9 + i * 3
                        ta = qpool.tile([P, nt], f32, name="ta", tag="ta")
                        tb = qpool.tile([P, nt], f32, name="tb", tag="tb")
                        nc.scalar.activation(
                            out=ta[:], in_=yh[:, :, 0], func=copy,
                            scale=xsb[:, c:c + 1])
                        nc.scalar.activation(
                            out=tb[:], in_=yh[:, :, 1], func=copy,
                            scale=xsb[:, c + 1:c + 2])
                        nc.gpsimd.tensor_tensor(
                            out=ta[:], in0=ta[:], in1=tb[:], op=add)
                        nc.scalar.activation(
                            out=tb[:], in_=yh[:, :, 2], func=copy,
                            scale=xsb[:, c + 2:c + 3])
                        nc.gpsimd.tensor_tensor(
                            out=ov[:, :, i], in0=ta[:], in1=tb[:], op=add)
                    if not light_store or g < 1:
                        dst = out[g * N_SUB:(g + 1) * N_SUB, :, :].rearrange(
                            "a (s t) i -> (a s) t i", s=SEGS
                        )[:, t0:t1, :]
                        nc.sync.dma_start(out=dst, in_=ot[:])

            if ops_mode == "full":
                # Software-pipelined emission: unit u+1's ACT products are
                # issued before unit u's accumulates so the unit-boundary
                # bubble overlaps. The last POOL_UNITS units run fully on
                # ACT+GPSIMD, decoupled from the DVE pipeline.
                dve_units = units[:len(units) - POOL_UNITS]
                pool_units = units[len(units) - POOL_UNITS:]
                if pool_units:
                    with (
                        tc.tile_pool(name="poolout", bufs=2) as ppool_,
                        tc.tile_pool(name="poolt", bufs=4) as qpool_,
                    ):
                        ppool, qpool = ppool_, qpool_
                        for g, h in pool_units:
                            emit_pool_unit(g, h)
                        prev = None
                        for u, (g, h) in enumerate(dve_units):
                            prod = emit_products(g, h)
                            if prev is not None:
                                emit_accums(*prev)
                            prev = (g, h, *prod)
                        if prev is not None:
                            emit_accums(*prev)
                else:
                    prev = None
                    for u, (g, h) in enumerate(dve_units):
                        prod = emit_products(g, h)
                        if prev is not None:
                            emit_accums(*prev)
                        prev = (g, h, *prod)
                    if prev is not None:
                        emit_accums(*prev)
                units = []

            for g, h in units:
                if True:
                    t0, t1 = bounds[h], bounds[h + 1]
                    nt = t1 - t0
                    ot = opool.tile([P, nt * 3], f32, name=f"ot{g}", tag="ot")
                    ov = ot.rearrange("p (t i) -> p t i", i=3)
                    yh = yv[:, t0:t1, :]
                    if ops_mode != "full":
                        # Engine-isolation probe modes (timing only; output
                        # values are garbage except ops_mode="none").
                        if ops_mode in ("none", "dve", "dve2", "pool",
                                        "sttu", "ttu"):
                            nc.vector.memset(ot[:], 0.0)
                        for i in range(3):
                            c = g * 9 + i * 3
                            if ops_mode == "act":
                                nc.scalar.activation(
                                    out=ov[:, :, i], in_=yh[:, :, 0], func=copy,
                                    scale=xsb[:, c:c + 1])
                            elif ops_mode == "dve":
                                nc.vector.scalar_tensor_tensor(
                                    out=ov[:, :, i], in0=yh[:, :, 1],
                                    scalar=xsb[:, c:c + 1], in1=ov[:, :, i],
                                    op0=mult, op1=add)
                            elif ops_mode == "dve2":
                                for _ in range(2):
                                    nc.vector.scalar_tensor_tensor(
                                        out=ov[:, :, i], in0=yh[:, :, 1],
                                        scalar=xsb[:, c:c + 1], in1=ov[:, :, i],
                                        op0=mult, op1=add)
                            elif ops_mode == "pool":
                                nc.gpsimd.tensor_tensor(
                                    out=ov[:, :, i], in0=ov[:, :, i],
                                    in1=yh[:, :, 2], op=add)
                            elif ops_mode == "tred":
                                nc.vector.tensor_reduce(
                                    out=ov[:, :, i], in_=yh,
                                    axis=mybir.AxisListType.X,
                                    op=add)
                            elif ops_mode == "tsmul":
                                nc.vector.tensor_scalar(
                                    out=ov[:, :, i], in0=yh[:, :, 1],
                                    scalar1=xsb[:, c:c + 1], scalar2=None,
                                    op0=mult)
                            elif ops_mode == "sttp":
                                w = 1024
                                pm = pspool.tile([P, w], f32, name="pm",
                                                 tag="pm")
                                nc.scalar.activation(
                                    out=pm[:], in_=yh[:, :w, 0], func=copy,
                                    scale=xsb[:, c:c + 1])
                                for _ in range(2):
                                    nc.vector.scalar_tensor_tensor(
                                        out=ov[:, :w, i], in0=yh[:, :w, 1],
                                        scalar=xsb[:, c + 1:c + 2], in1=pm[:],
                                        op0=mult, op1=add)
                            elif ops_mode == "sttq":
                                w = 1024
                                qm = tpool.tile([P, w], f32, name="qm",
                                                tag="qm")
                                nc.scalar.activation(
                                    out=qm[:], in_=yh[:, :w, 0], func=copy,
                                    scale=xsb[:, c:c + 1])
                                for _ in range(2):
                                    nc.vector.scalar_tensor_tensor(
                                        out=ov[:, :w, i], in0=yh[:, :w, 1],
                                        scalar=xsb[:, c + 1:c + 2], in1=qm[:],
                                        op0=mult, op1=add)
                            elif ops_mode in ("sttu", "sttm", "copys", "ttu"):
                                tm = tpool.tile([P, nt], f32, name="tm",
                                                tag="tm")
                                nc.vector.tensor_scalar(
                                    out=tm[:], in0=yh[:, :, 1],
                                    scalar1=xsb[:, c:c + 1], scalar2=None,
                                    op0=mult)
                                if ops_mode == "sttu":
                                    nc.vector.scalar_tensor_tensor(
                                        out=tm[:], in0=tm[:],
                                        scalar=xsb[:, c:c + 1], in1=tm[:],
                                        op0=mult, op1=add)
                                elif ops_mode == "sttm":
                                    nc.vector.scalar_tensor_tensor(
                                        out=ov[:, :, i], in0=tm[:],
                                        scalar=xsb[:, c:c + 1], in1=tm[:],
                                        op0=mult, op1=add)
                                elif ops_mode == "copys":
                                    nc.vector.tensor_copy(
                                        out=ov[:, :, i], in_=tm[:])
                                elif ops_mode == "ttu":
                                    nc.vector.tensor_tensor(
                                        out=tm[:], in0=tm[:], in1=tm[:],
                                        op=add)
                        store_unit(g, h, ot)
    nc.compile()
    return nc


def kernel(x: np.ndarray, y: np.ndarray) -> np.ndarray:
    global LAST, _CACHED_NC
    x = np.ascontiguousarray(x, dtype=np.float32)
    y = np.ascontiguousarray(y, dtype=np.float32)
    assert x.shape == (4, 64, 3, 3) and y.shape == (4, 100000, 3)

    if _CACHED_NC is None:
        _CACHED_NC = build_bass()
    nc = _CACHED_NC

    x_flat = x.reshape(256, 3, 3)
    in_maps = []
    for c in range(N_CORES):
        b = c // 2
        xl = x_flat[c * N_PER_CORE:(c + 1) * N_PER_CORE]  # [32, 3, 3]
        per_a = xl.reshape(N_GROUPS, N_SUB, 9).transpose(1, 0, 2).reshape(N_SUB, 72)
        xsb_np = np.ascontiguousarray(np.repeat(per_a, SEGS, axis=0))  # [128, 72]
        in_maps.append({"xs": xsb_np, "ys": y[b]})

    res = run_bass_kernel_spmd(
        nc, in_maps, core_ids=list(range(N_CORES)), trace=TRACE,
    )
    LAST = res
    out = np.concatenate([r["out"] for r in res.results], axis=0)
    return out.reshape(4, 64, 100000, 3)


def _make_in_maps(x, y):
    x_flat = x.reshape(256, 3, 3)
    in_maps = []
    for c in range(N_CORES):
        b = c // 2
        xl = x_flat[c * N_PER_CORE:(c + 1) * N_PER_CORE]
        per_a = xl.reshape(N_GROUPS, N_SUB, 9).transpose(1, 0, 2).reshape(N_SUB, 72)
        xsb_np = np.ascontiguousarray(np.repeat(per_a, SEGS, axis=0))
        in_maps.append({"xs": xsb_np, "ys": y[b]})
    return in_maps


def _prepare_exec(nc, in_maps):
    """Build a jitted 8-core executor for `nc` with device-resident inputs.

    Returns (run_once, ins_dev, zeros) where run_once(outs) executes the
    NEFF once per core and returns new device outputs (pass them back in as
    the donated output buffers for the next call)."""
    import jax
    import concourse.mybir as mybir_
    from jax.experimental.shard_map import shard_map
    from jax.sharding import Mesh, NamedSharding, PartitionSpec
    from concourse.bass2jax import (
        _bass_exec_p, install_neuronx_cc_hook, partition_id_tensor,
    )

    install_neuronx_cc_hook()
    partition_name = nc.partition_id_tensor.name if nc.partition_id_tensor else None
    in_names, out_names, out_avals, zero_outs = [], [], [], []
    for alloc in nc.m.functions[0].allocations:
        if not isinstance(alloc, mybir_.MemoryLocationSet):
            continue
        name = alloc.memorylocations[0].name
        if alloc.kind == "ExternalInput":
            if name != partition_name:
                in_names.append(name)
        elif alloc.kind == "ExternalOutput":
            shape = tuple(alloc.tensor_shape)
            dtype = mybir_.dt.np(alloc.dtype)
            out_names.append(name)
            out_avals.append(jax.core.ShapedArray(shape, dtype))
            zero_outs.append(np.zeros(shape, dtype))
    n_params = len(in_names)
    n_outs = len(out_names)
    all_names = in_names + out_names + ([partition_name] if partition_name else [])

    def _body(*args):
        operands = list(args)
        if partition_name is not None:
            operands.append(partition_id_tensor())
        outs = _bass_exec_p.bind(
            *operands,
            out_avals=tuple(out_avals),
            in_names=tuple(all_names),
            out_names=tuple(out_names),
            lowering_input_output_aliases=(),
            sim_require_finite=True,
            sim_require_nnan=True,
            nc=nc,
        )
        return tuple(outs)

    devices = jax.devices()[:N_CORES]
    mesh = Mesh(np.asarray(devices), ("core",))
    spec = PartitionSpec("core")
    sharded = jax.jit(
        shard_map(
            _body, mesh=mesh, in_specs=(spec,) * (n_params + n_outs),
            out_specs=(spec,) * n_outs, check_rep=False,
        ),
        donate_argnums=tuple(range(n_params, n_params + n_outs)),
        keep_unused=True,
    )
    sh = NamedSharding(mesh, spec)
    ins_dev = [
        jax.device_put(
            np.concatenate([np.asarray(m[name]) for m in in_maps], axis=0), sh
        )
        for name in in_names
    ]
    zeros = [
        jax.device_put(
            np.zeros((N_CORES * z.shape[0], *z.shape[1:]), z.dtype), sh
        )
        for z in zero_outs
    ]

    def run_once(outs):
        res = sharded(*ins_dev, *outs)
        jax.block_until_ready(res)
        return list(res)

    return run_once, zeros


def bench(x, y, reps_pair=(9, 65), samples=24, ops_mode="full"):
    """Measure steady-state per-workload HW time by differencing kernels
    that run the workload `reps_pair[0]` vs `reps_pair[1]` times."""
    import time
    x = np.ascontiguousarray(x, dtype=np.float32)
    y = np.ascontiguousarray(y, dtype=np.float32)
    in_maps = _make_in_maps(x, y)
    times = {}
    for reps in reps_pair:
        nc = build_bass(reps=reps, ops_mode=ops_mode)
        run_once, zeros = _prepare_exec(nc, in_maps)
        outs = run_once(zeros)  # compile + warm
        ts = []
        for _ in range(samples):
            t0 = time.perf_counter()
            outs = run_once(outs)
            ts.append(time.perf_counter() - t0)
        ts.sort()
        times[reps] = ts[len(ts) // 2]  # median: bimodal fast-path outliers
        print(f"reps={reps}: med {times[reps]*1e3:.2f} ms  min {ts[0]*1e3:.2f}  "
              f"all {[f'{t*1e3:.1f}' for t in ts]}")
    r1, r2 = reps_pair
    per_iter_s = (times[r2] - times[r1]) / (r2 - r1)
    return per_iter_s * 1e9



# revision 12
# speedup vs baseline: 1.9120x; 1.9120x over previous
"""Trainium2 Bass kernel for nn_BatchMatMulModule.

Computes out = einsum("bnij,bmj->bnmi", x, y) with
  x: [4, 64, 3, 3] f32, y: [4, 100000, 3] f32 -> out: [4, 64, 100000, 3] f32.

The output (307 MB) dwarfs the inputs (4.8 MB), so the kernel is bound by
HBM write bandwidth (~358 GB/s per NeuronCore => ~107 us floor for the
38.4 MB each core stores). Strategy (v2, TensorE-based):

- Shard the 256 flat (b, n) pairs across 8 cores: core c handles b = c // 2
  and 32 consecutive n. Output slice [32, 100000, 3] is contiguous in DRAM.
- SBUF/PSUM layout: partition p = a * 32 + s with a = n % 4 (within a group
  of 4 n's) and s = m-segment (100000 = 32 segments x 3125 rows).
- The contraction out[(a,s), t, i] = sum_j x[n,i,j] * y[(s,t),j] is run on
  the idle TensorE as a matmul with a block-diagonal stationary operand:
    W_{g,i}[(s',j), (a,s)] = x[g*4+a, i, j] * delta(s,s')   [96 x 128]
    Y[(s',j), t] = y[s'*3125 + t, j]                        [96 x 3125]
    psum_i = W_{g,i}.T @ Y[:, t0:t1]                        [128 x <=512]
  Operands are fp32 bitcast to float32r (FP22-truncated): full PE rate at
  N >= 256, rel err ~1e-4 (harness gate is 2e-2).
- PSUM cannot be DMA'd, so the mandatory PSUM->SBUF evacuation doubles as
  the i-interleave: one ACT/DVE copy per chunk reads the 3 psum planes
  (strided) and writes the (t, i)-interleaved SBUF tile (contiguous), which
  then stores as a 768 KB contiguous-per-partition DMA.
- Engine budget per core: DMA-out ~107 us (bottleneck), PE ~25 us,
  ACT/DVE alternate evacuation chunks at ~40 us each.
"""

import numpy as np

import concourse.bacc as bacc
import concourse.mybir as mybir
from concourse.bass_utils import run_bass_kernel_spmd
from concourse.tile import TileContext

N_CORES = 8
P = 128
N_PER_CORE = 32   # n per core
N_SUB = 4         # a: n's packed per partition-group
N_GROUPS = N_PER_CORE // N_SUB  # 8 groups, one weight set each
SEGS = P // N_SUB               # 32 m-segments
M = 100000
ROWS = M // SEGS                # 3125 t-rows per segment
K = SEGS * 3                    # 96 contraction rows (s', j)

CHUNK = 512                     # t-chunk = one PSUM bank of fp32
CHUNK_STARTS = list(range(0, ROWS, CHUNK))
ROWS_PAD = ROWS + (ROWS % 2)    # fp32r matmul needs even free sizes; pad tail

TRACE = False
LAST = None  # last BassKernelResults, for test harness introspection

_CACHED_NC = None

# Tuning knobs.
EVAC_MODE = "fused"  # "fused": 1 copy/chunk (strided psum read, contig write)
                     # "plane3": 3 copies/chunk (contig read, strided write)
OUT_BUFS = 3
PSUM_BUFS = 2
Y_SPLITS = 4


def build_bass(reps: int = 1, ops_mode: str = "full"):
    nc = bacc.Bacc(
        "TRN2",
        debug=False,
        enable_asserts=False,
        target_bir_lowering=False,
        num_devices=N_CORES,
    )
    f32 = mybir.dt.float32
    f32r = mybir.dt.float32r
    copy = mybir.ActivationFunctionType.Copy

    # Host-prearranged inputs (float32r: fp32 bytes, FP22-truncated by the PE):
    #  xw[k, (g*3+i)*128 + a*32 + s] = x[g*4+a, i, j] * delta(s, k//3), j=k%3
    #  ys[k, t] = y[b, (k//3)*3125 + t, k%3]
    xw = nc.dram_tensor("xw", [K, N_GROUPS * 3 * P], f32r, kind="ExternalInput").ap()
    ys = nc.dram_tensor("ys", [K, ROWS_PAD], f32r, kind="ExternalInput").ap()
    out = nc.dram_tensor("out", [N_PER_CORE, M, 3], f32, kind="ExternalOutput").ap()

    with TileContext(nc) as tc:
        with (
            tc.tile_pool(name="const", bufs=1) as cpool,
            tc.tile_pool(name="outp", bufs=OUT_BUFS) as opool,
            tc.tile_pool(name="psp", bufs=PSUM_BUFS, space="PSUM") as pspool,
        ):
            wt = cpool.tile([K, N_GROUPS * 3 * P], f32r)
            # First group's weights first so g=0 matmuls start early.
            nc.sync.dma_start(out=wt[:, : 3 * P], in_=xw[:, : 3 * P])
            nc.sync.dma_start(out=wt[:, 3 * P:], in_=xw[:, 3 * P:])

            y_tile = cpool.tile([K, ROWS_PAD], f32r)
            yb = [ROWS_PAD * h // Y_SPLITS for h in range(Y_SPLITS + 1)]
            for h in range(Y_SPLITS):
                nc.sync.dma_start(
                    out=y_tile[:, yb[h]:yb[h + 1]], in_=ys[:, yb[h]:yb[h + 1]]
                )

            out_r = out.rearrange("(g a) (s t) i -> g a s t i", a=N_SUB, s=SEGS)

            def emit_unit(g, c, u):
                t0 = CHUNK_STARTS[c]
                nt = min(CHUNK, ROWS - t0)           # rows actually stored
                nt_mm = nt + (nt % 2)                # fp32r: even matmul width
                pst = pspool.tile([P, 3 * CHUNK], f32, name="ps", tag="ps")
                for i in range(3):
                    blk = (g * 3 + i) * P
                    nc.tensor.matmul(
                        pst[:, i * CHUNK: i * CHUNK + nt_mm],
                        wt[:, blk: blk + P],
                        y_tile[:, t0: t0 + nt_mm],
                        start=True,
                        stop=True,
                    )
                ot = opool.tile([P, 3 * CHUNK], f32, name="ot", tag="ot")
                # psum viewed [p, i, t] (planes at CHUNK spacing), out (t, i).
                psv = pst.rearrange("p (i t) -> p i t", i=3)[:, :, :nt]
                if EVAC_MODE == "fused":
                    src = psv.rearrange("p i t -> p t i")
                    dst_sb = ot.rearrange("p (t i) -> p t i", i=3)[:, :nt, :]
                    if u % 2 == 0:
                        nc.scalar.activation(out=dst_sb, in_=src, func=copy)
                    else:
                        nc.vector.tensor_copy(out=dst_sb, in_=src)
                else:
                    ov = ot.rearrange("p (t i) -> p t i", i=3)[:, :nt, :]
                    for i in range(3):
                        act = (i + u) % 3 != 0
                        if act:
                            nc.scalar.activation(
                                out=ov[:, :, i], in_=psv[:, i, :], func=copy)
                        else:
                            nc.vector.tensor_copy(
                                out=ov[:, :, i], in_=psv[:, i, :])
                dst = out_r[g][:, :, t0: t0 + nt, :].rearrange(
                    "a s t i -> (a s) t i")
                nc.sync.dma_start(out=dst, in_=ot[:, : 3 * nt])

            u = 0
            for _ in range(reps):
                for g in range(N_GROUPS):
                    for c in range(len(CHUNK_STARTS)):
                        emit_unit(g, c, u)
                        u += 1
    nc.compile()
    return nc


def _make_in_maps(x, y):
    x_flat = x.reshape(256, 3, 3)
    sr = np.arange(SEGS)
    in_maps = []
    for c in range(N_CORES):
        b = c // 2
        xg = x_flat[c * N_PER_CORE:(c + 1) * N_PER_CORE].reshape(
            N_GROUPS, N_SUB, 3, 3)  # [g, a, i, j]
        wall = np.zeros((SEGS, 3, N_GROUPS, 3, N_SUB, SEGS), np.float32)
        wall[sr, :, :, :, :, sr] = xg.transpose(3, 0, 2, 1)[None]
        xw_np = np.ascontiguousarray(wall.reshape(K, N_GROUPS * 3 * P))
        ys_np = np.zeros((K, ROWS_PAD), np.float32)
        ys_np[:, :ROWS] = (
            y[b].reshape(SEGS, ROWS, 3).transpose(0, 2, 1).reshape(K, ROWS))
        in_maps.append({"xw": xw_np, "ys": ys_np})
    return in_maps


def kernel(x: np.ndarray, y: np.ndarray) -> np.ndarray:
    global LAST, _CACHED_NC
    x = np.ascontiguousarray(x, dtype=np.float32)
    y = np.ascontiguousarray(y, dtype=np.float32)
    assert x.shape == (4, 64, 3, 3) and y.shape == (4, 100000, 3)

    if _CACHED_NC is None:
        _CACHED_NC = build_bass()
    nc = _CACHED_NC

    in_maps = _make_in_maps(x, y)
    res = run_bass_kernel_spmd(
        nc, in_maps, core_ids=list(range(N_CORES)), trace=TRACE,
    )
    LAST = res
    out = np.concatenate([r["out"] for r in res.results], axis=0)
    return out.reshape(4, 64, 100000, 3)


def _prepare_exec(nc, in_maps):
    """Build a jitted 8-core executor for `nc` with device-resident inputs.

    Returns (run_once, zeros) where run_once(outs) executes the NEFF once per
    core and returns new device outputs (pass them back in as the donated
    output buffers for the next call)."""
    import jax
    import concourse.mybir as mybir_
    from jax.experimental.shard_map import shard_map
    from jax.sharding import Mesh, NamedSharding, PartitionSpec
    from concourse.bass2jax import (
        _bass_exec_p, install_neuronx_cc_hook, partition_id_tensor,
    )

    install_neuronx_cc_hook()
    partition_name = nc.partition_id_tensor.name if nc.partition_id_tensor else None
    in_names, out_names, out_avals, zero_outs = [], [], [], []
    for alloc in nc.m.functions[0].allocations:
        if not isinstance(alloc, mybir_.MemoryLocationSet):
            continue
        name = alloc.memorylocations[0].name
        if alloc.kind == "ExternalInput":
            if name != partition_name:
                in_names.append(name)
        elif alloc.kind == "ExternalOutput":
            shape = tuple(alloc.tensor_shape)
            dtype = mybir_.dt.np(alloc.dtype)
            out_names.append(name)
            out_avals.append(jax.core.ShapedArray(shape, dtype))
            zero_outs.append(np.zeros(shape, dtype))
    n_params = len(in_names)
    n_outs = len(out_names)
    all_names = in_names + out_names + ([partition_name] if partition_name else [])

    def _body(*args):
        operands = list(args)
        if partition_name is not None:
            operands.append(partition_id_tensor())
        outs = _bass_exec_p.bind(
            *operands,
            out_avals=tuple(out_avals),
            in_names=tuple(all_names),
            out_names=tuple(out_names),
            lowering_input_output_aliases=(),
            sim_require_finite=True,
            sim_require_nnan=True,
            nc=nc,
        )
        return tuple(outs)

    devices = jax.devices()[:N_CORES]
    mesh = Mesh(np.asarray(devices), ("core",))
    spec = PartitionSpec("core")
    sharded = jax.jit(
        shard_map(
            _body, mesh=mesh, in_specs=(spec,) * (n_params + n_outs),
            out_specs=(spec,) * n_outs, check_rep=False,
        ),
        donate_argnums=tuple(range(n_params, n_params + n_outs)),
        keep_unused=True,
    )
    sh = NamedSharding(mesh, spec)
    ins_dev = [
        jax.device_put(
            np.concatenate([np.asarray(m[name]) for m in in_maps], axis=0), sh
        )
        for name in in_names
    ]
    zeros = [
        jax.device_put(
            np.zeros((N_CORES * z.shape[0], *z.shape[1:]), z.dtype), sh
        )
        for z in zero_outs
    ]

    def run_once(outs):
        res = sharded(*ins_dev, *outs)
        jax.block_until_ready(res)
        return list(res)

    return run_once, zeros


def bench(x, y, reps_pair=(9, 65), samples=24, ops_mode="full"):
    """Measure steady-state per-workload HW time by differencing kernels
    that run the workload `reps_pair[0]` vs `reps_pair[1]` times."""
    import time
    x = np.ascontiguousarray(x, dtype=np.float32)
    y = np.ascontiguousarray(y, dtype=np.float32)
    in_maps = _make_in_maps(x, y)
    times = {}
    for reps in reps_pair:
        nc = build_bass(reps=reps, ops_mode=ops_mode)
        run_once, zeros = _prepare_exec(nc, in_maps)
        outs = run_once(zeros)  # compile + warm
        ts = []
        for _ in range(samples):
            t0 = time.perf_counter()
            outs = run_once(outs)
            ts.append(time.perf_counter() - t0)
        ts.sort()
        times[reps] = ts[len(ts) // 2]  # median: bimodal fast-path outliers
        print(f"reps={reps}: med {times[reps]*1e3:.2f} ms  min {ts[0]*1e3:.2f}  "
              f"all {[f'{t*1e3:.1f}' for t in ts]}")
    r1, r2 = reps_pair
    per_iter_s = (times[r2] - times[r1]) / (r2 - r1)
    return per_iter_s * 1e9
